# revision 1
# baseline (speedup 1.0000x reference)
"""Llama decode block (single token) on 8 TRN2 NeuronCores, tensor-parallel.

Sharding (per sharding_hint): w_q/w_k/w_v/w_ff1 column-sharded, w_o/w_ff2
row-sharded, KV cache sharded by head (4 heads/core). AllReduce after the
attention output projection and after w_ff2. The residual stream x is folded
into the all-reduces as x/8 per core, so each AR output is the full residual
sum directly.

Per-core dataflow (all matvecs run on the PE with the activation vector as
the stationary operand, streaming the weights as the moving operand):
  rmsnorm(x) -> h_cols[128,32]  (PE transpose of [32,128] rows)
  q/k/v[1,512] = h @ W         (32 k-blocks accumulated in PSUM)
  RoPE on q,k rows; q pre-scaled by 1/sqrt(128)
  scores: per 128-token tile, fused DVE multiply+reduce against K tiles
  softmax without max-subtraction (scores are O(8), exp is safe in f32)
  attn@V: per-tile PE matmuls, V tile stationary -> o[128(d),4(h)] cols
  o @ w_o + x/8 -> [1,4096] -> AllReduce #1 -> x2
  rmsnorm(x2) -> h2_cols; silu(h2 @ w_ff1) -> a[128,11] cols
  a @ w_ff2 + x2/8 -> [1,4096] -> AllReduce #2 -> output
"""

import math

import numpy as np

import concourse.bass as bass
import concourse.mybir as mybir
import concourse.tile as tile
from concourse import bacc
from concourse import bass_utils

F32 = mybir.dt.float32
AF = mybir.ActivationFunctionType
ALU = mybir.AluOpType

HIDDEN = 4096
N_HEADS = 32
HEAD_DIM = 128
INTERM = 11008
KV_LEN = 4096
N_CORES = 8

HEADS_PC = N_HEADS // N_CORES          # 4 heads per core
QKV_N = HEADS_PC * HEAD_DIM            # 512
FF_N = INTERM // N_CORES               # 1376
KB = HIDDEN // 128                     # 32 k-blocks of the hidden dim
T_TILES = KV_LEN // 128                # 32 token tiles
FF_KB_SIZES = [128] * 10 + [96]        # 1376 = 10*128 + 96
SCALE = 1.0 / math.sqrt(HEAD_DIM)


def _emit(nc, tc):
    i = {}  # dram input handles

    def din(name, shape):
        i[name] = nc.dram_tensor(name, list(shape), F32, kind="ExternalInput").ap()

    din("x", [HIDDEN])
    din("attn_norm", [HIDDEN])
    din("ffn_norm", [HIDDEN])
    din("sin", [HEAD_DIM // 2])
    din("ident32", [32, 32])
    din("cos", [HEAD_DIM // 2])
    din("wq", [HIDDEN, QKV_N])
    din("wk", [HIDDEN, QKV_N])
    din("wv", [HIDDEN, QKV_N])
    din("wo", [QKV_N, HIDDEN])
    din("kc", [KV_LEN, QKV_N])
    din("vc", [KV_LEN, QKV_N])
    din("wf1", [HIDDEN, FF_N])
    din("wf2", [FF_N, HIDDEN])
    y = nc.dram_tensor("y", [HIDDEN], F32, kind="ExternalOutput").ap()

    with (
        tc.tile_pool(name="const", bufs=1) as cpool,
        tc.tile_pool(name="wbig", bufs=4) as wpool,
        tc.tile_pool(name="kpool", bufs=2) as kpool,
        tc.tile_pool(name="vpool", bufs=2) as vpool,
        tc.tile_pool(name="sm", bufs=1) as sm,
        tc.tile_pool(name="scr", bufs=4) as scr,
        tc.tile_pool(name="psum", bufs=8, space="PSUM") as pp,
        tc.tile_pool(name="dram", bufs=1, space="DRAM") as dram,
    ):
        # ---- constants ----
        ones32 = cpool.tile([32, 1], F32)
        nc.vector.memset(ones32[:], 1.0)
        ones128 = cpool.tile([128, 1], F32)
        nc.vector.memset(ones128[:], 1.0)
        eighth = cpool.tile([1, 1], F32)
        nc.vector.memset(eighth[:], 1.0 / N_CORES)
        eps11 = cpool.tile([1, 1], F32)
        nc.vector.memset(eps11[:], 1e-6)
        ident32 = cpool.tile([32, 32], F32)
        nc.sync.dma_start(ident32[:], i["ident32"])
        ones_r32 = cpool.tile([1, 32], F32)
        nc.vector.memset(ones_r32[:], 1.0)
        ones_r128 = cpool.tile([1, 128], F32)
        nc.vector.memset(ones_r128[:], 1.0)

        sin_sb = cpool.tile([1, 64], F32)
        cos_sb = cpool.tile([1, 64], F32)
        nc.sync.dma_start(sin_sb[:], i["sin"].rearrange("(a d) -> a d", a=1))
        nc.sync.dma_start(cos_sb[:], i["cos"].rearrange("(a d) -> a d", a=1))
        sinq = cpool.tile([1, 64], F32)
        cosq = cpool.tile([1, 64], F32)
        nc.vector.tensor_scalar_mul(sinq[:], sin_sb[:], SCALE)
        nc.vector.tensor_scalar_mul(cosq[:], cos_sb[:], SCALE)

        # ---- rmsnorm #1 -> h_cols [128, 32] ----
        def rmsnorm_cols(x_dram, norm_dram, tag):
            x_rows = sm.tile([32, 128], F32, name=f"x_rows_{tag}", tag="x_rows")
            nrm_rows = sm.tile([32, 128], F32, name=f"nrm_rows_{tag}", tag="nrm_rows")
            nc.sync.dma_start(x_rows[:], x_dram.rearrange("(a d) -> a d", a=32))
            nc.sync.dma_start(nrm_rows[:], norm_dram.rearrange("(a d) -> a d", a=32))
            sq = sm.tile([32, 128], F32, name=f"sq_{tag}", tag="sq")
            ssq = sm.tile([32, 1], F32, name=f"ssq_{tag}", tag="ssq")
            nc.scalar.activation(sq[:], x_rows[:], AF.Square, accum_out=ssq[:])
            ms_psum = pp.tile([1, 1], F32, name=f"ms_psum_{tag}", tag="ps")
            nc.tensor.matmul(ms_psum[:], ones32[:], ssq[:])
            rstd = sm.tile([1, 1], F32, name=f"rstd_{tag}", tag="rstd")
            # sqrt(mean + eps), then reciprocal (Rsqrt activation is banned)
            nc.scalar.activation(rstd[:], ms_psum[:], AF.Sqrt,
                                 bias=eps11[:], scale=1.0 / HIDDEN)
            nc.vector.reciprocal(rstd[:], rstd[:])
            rstd_ps = pp.tile([32, 1], F32, name=f"rstd_ps_{tag}", tag="ps")
            nc.tensor.matmul(rstd_ps[:], ones_r32[:], rstd[:])
            rstd32 = sm.tile([32, 1], F32, name=f"rstd32_{tag}", tag="rstd32")
            nc.vector.tensor_copy(rstd32[:], rstd_ps[:])
            h_rows = sm.tile([32, 128], F32, name=f"h_rows_{tag}", tag="h_rows")
            nc.vector.tensor_tensor(h_rows[:], x_rows[:], nrm_rows[:], ALU.mult)
            nc.vector.tensor_scalar_mul(h_rows[:], h_rows[:], rstd32[:])
            h_psum = pp.tile([128, 32], F32, name=f"h_psum_{tag}", tag="ps")
            nc.tensor.transpose(h_psum[:], h_rows[:], ident32[:])
            h_cols = sm.tile([128, 32], F32, name=f"h_cols_{tag}", tag="hcols")
            nc.vector.tensor_copy(h_cols[:], h_psum[:])
            return h_cols

        h_cols = rmsnorm_cols(i["x"], i["attn_norm"], "a")

        # ---- q/k/v = h @ W (h stationary, weights moving) ----
        qkv_rows = {}
        for wname in ("wq", "wk", "wv"):
            ps = pp.tile([1, QKV_N], F32, name=f"ps_{wname}", tag="ps")
            for t in range(4):
                wt = wpool.tile([128, 8, 512], F32, name=f"{wname}_t", tag="w")
                nc.sync.dma_start(
                    wt[:],
                    i[wname][t * 1024:(t + 1) * 1024, :].rearrange(
                        "(b p) c -> p b c", p=128),
                )
                for b in range(8):
                    kb = t * 8 + b
                    nc.tensor.matmul(
                        ps[:], h_cols[:, kb:kb + 1], wt[:, b, :],
                        start=(kb == 0), stop=(kb == KB - 1),
                    )
            row = sm.tile([1, QKV_N], F32, name=f"{wname}_row")
            nc.scalar.copy(row[:], ps[:])
            qkv_rows[wname] = row

        # ---- RoPE on q (pre-scaled by 1/sqrt(d)) and k ----
        def rope(row, cos_t, sin_t, tag):
            out = sm.tile([1, QKV_N], F32, name=f"rope_{tag}")
            tmp = sm.tile([1, QKV_N], F32, name=f"rope_tmp_{tag}")
            r3 = row[:].rearrange("a (h d) -> a h d", h=HEADS_PC)
            o3 = out[:].rearrange("a (h d) -> a h d", h=HEADS_PC)
            t3 = tmp[:].rearrange("a (h d) -> a h d", h=HEADS_PC)
            x1, x2 = r3[:, :, 0:64], r3[:, :, 64:128]
            cb = cos_t[:].unsqueeze(1).to_broadcast((1, HEADS_PC, 64))
            sb = sin_t[:].unsqueeze(1).to_broadcast((1, HEADS_PC, 64))
            nc.vector.tensor_tensor(o3[:, :, 0:64], x1, cb, ALU.mult)
            nc.vector.tensor_tensor(t3[:, :, 0:64], x2, sb, ALU.mult)
            nc.vector.tensor_sub(o3[:, :, 0:64], o3[:, :, 0:64], t3[:, :, 0:64])
            nc.vector.tensor_tensor(o3[:, :, 64:128], x2, cb, ALU.mult)
            nc.vector.tensor_tensor(t3[:, :, 64:128], x1, sb, ALU.mult)
            nc.vector.tensor_add(o3[:, :, 64:128], o3[:, :, 64:128],
                                 t3[:, :, 64:128])
            return out

        q_rot = rope(qkv_rows["wq"], cosq, sinq, "q")
        k_rot = rope(qkv_rows["wk"], cos_sb, sin_sb, "k")
        v_row = qkv_rows["wv"]

        q_rep = sm.tile([128, QKV_N], F32, name="q_rep")
        qrep_ps = pp.tile([128, QKV_N], F32, name="qrep_ps", tag="ps")
        nc.tensor.matmul(qrep_ps[:], ones_r128[:], q_rot[:])
        nc.vector.tensor_copy(q_rep[:], qrep_ps[:])

        # ---- attention over the KV cache ----
        o_psum = pp.tile([128, HEADS_PC], F32, name="o_psum", tag="ps")
        denom_acc = sm.tile([128, HEADS_PC], F32, name="denom_acc")
        nc.vector.memset(denom_acc[:], 0.0)

        for st in range(4):
            k_sup = kpool.tile([128, 8, 512], F32, name="k_sup", tag="k")
            v_sup = vpool.tile([128, 8, 512], F32, name="v_sup", tag="v")
            nc.sync.dma_start(
                k_sup[:],
                i["kc"][st * 1024:(st + 1) * 1024, :].rearrange(
                    "(b p) c -> p b c", p=128),
            )
            nc.sync.dma_start(
                v_sup[:],
                i["vc"][st * 1024:(st + 1) * 1024, :].rearrange(
                    "(b p) c -> p b c", p=128),
            )
            for b in range(8):
                gt = st * 8 + b
                scores = scr.tile([128, HEADS_PC], F32, name="scores", tag="sc")
                scratch = scr.tile([128, QKV_N], F32, name="scratch", tag="scratch")
                nc.vector.tensor_tensor(scratch[:], k_sup[:, b, :], q_rep[:],
                                        ALU.mult)
                nc.vector.tensor_reduce(
                    scores[:],
                    scratch[:].rearrange("p (h d) -> p h d", h=HEADS_PC),
                    mybir.AxisListType.X, ALU.add)
                expt = scr.tile([128, HEADS_PC], F32, name="expt", tag="expt")
                nc.scalar.activation(expt[:], scores[:], AF.Exp)
                nc.vector.tensor_add(denom_acc[:], denom_acc[:], expt[:])
                for h in range(HEADS_PC):
                    # start clears has_written for the whole PSUM bank, so
                    # only the very first matmul into o_psum may set it.
                    nc.tensor.matmul(
                        o_psum[:, h:h + 1],
                        v_sup[:, b, h * 128:(h + 1) * 128],
                        expt[:, h:h + 1],
                        start=(gt == 0 and h == 0), stop=False,
                    )

        # current-token contribution (position KV_LEN)
        s_new = sm.tile([1, HEADS_PC], F32, name="s_new")
        scr_new = sm.tile([1, QKV_N], F32, name="scr_new")
        nc.vector.tensor_tensor(scr_new[:], q_rot[:], k_rot[:], ALU.mult)
        nc.vector.tensor_reduce(
            s_new[:],
            scr_new[:].rearrange("a (h d) -> a h d", h=HEADS_PC),
            mybir.AxisListType.X, ALU.add)
        e_new = sm.tile([1, HEADS_PC], F32, name="e_new")
        nc.scalar.activation(e_new[:], s_new[:], AF.Exp)
        for h in range(HEADS_PC):
            nc.tensor.matmul(
                o_psum[:, h:h + 1],
                v_row[:, h * 128:(h + 1) * 128],
                e_new[:, h:h + 1],
                start=False, stop=(h == HEADS_PC - 1),
            )

        denom_psum = pp.tile([1, HEADS_PC], F32, name="denom_psum", tag="ps")
        nc.tensor.matmul(denom_psum[:], ones128[:], denom_acc[:])
        denom = sm.tile([1, HEADS_PC], F32, name="denom")
        nc.vector.tensor_copy(denom[:], denom_psum[:])
        nc.vector.tensor_add(denom[:], denom[:], e_new[:])
        nc.vector.reciprocal(denom[:], denom[:])
        recip_ps = pp.tile([128, HEADS_PC], F32, name="recip_ps", tag="ps")
        nc.tensor.matmul(recip_ps[:], ones_r128[:], denom[:])
        recip_bc = sm.tile([128, HEADS_PC], F32, name="recip_bc")
        nc.vector.tensor_copy(recip_bc[:], recip_ps[:])
        o_sb = sm.tile([128, HEADS_PC], F32, name="o_sb")
        nc.vector.tensor_tensor(o_sb[:], o_psum[:], recip_bc[:], ALU.mult)

        # ---- o @ w_o + x/8 -> [1,4096] -> AllReduce #1 ----
        x_row = sm.tile([1, HIDDEN], F32, name="x_row", tag="xrow")
        nc.sync.dma_start(x_row[:], i["x"].rearrange("(a d) -> a d", a=1))

        chunks1 = [pp.tile([1, 512], F32, name=f"c1_{n}", tag="ps")
                   for n in range(8)]
        for kb in range(HEADS_PC):
            wo_t = wpool.tile([128, HIDDEN], F32, name="wo_t", tag="w")
            nc.sync.dma_start(wo_t[:], i["wo"][kb * 128:(kb + 1) * 128, :])
            for n in range(8):
                nc.tensor.matmul(
                    chunks1[n][:], o_sb[:, kb:kb + 1],
                    wo_t[:, n * 512:(n + 1) * 512],
                    start=(kb == 0), stop=False,
                )
        o_row = sm.tile([1, HIDDEN], F32, name="o_row", tag="outrow")
        for n in range(8):
            nc.tensor.matmul(
                chunks1[n][:], eighth[:], x_row[:, n * 512:(n + 1) * 512],
                start=False, stop=True,
            )
            nc.scalar.copy(o_row[:, n * 512:(n + 1) * 512], chunks1[n][:])

        ar1_in = dram.tile([HIDDEN], F32, name="ar1_in")
        ar1_out = dram.tile([HIDDEN], F32, name="ar1_out")
        nc.sync.dma_start(ar1_in[:], o_row[:])
        nc.gpsimd.collective_compute(
            "AllReduce", ALU.add,
            replica_groups=[list(range(N_CORES))],
            ins=[ar1_in[:].opt()], outs=[ar1_out[:].opt()],
        )

        # ---- MLP ----
        h2_cols = rmsnorm_cols(ar1_out[:], i["ffn_norm"], "b")
        x2_row = sm.tile([1, HIDDEN], F32, name="x2_row", tag="xrow")
        nc.sync.dma_start(x2_row[:], ar1_out[:].rearrange("(a d) -> a d", a=1))

        # two tiles (separate banks): start/stop must cover a consistent
        # partition count per zero region, and the 96-row tail block differs.
        f1a = pp.tile([128, 10], F32, name="f1a", tag="ps")
        f1b = pp.tile([96, 1], F32, name="f1b", tag="ps")
        for t in range(16):
            wt = wpool.tile([128, 2, FF_N], F32, name="wf1_t", tag="w")
            nc.sync.dma_start(
                wt[:],
                i["wf1"][t * 256:(t + 1) * 256, :].rearrange(
                    "(b p) c -> p b c", p=128),
            )
            for half in range(2):
                kb = 2 * t + half
                for mb in range(11):
                    sz = FF_KB_SIZES[mb]
                    out = f1a[:, mb:mb + 1] if mb < 10 else f1b[:]
                    nc.tensor.matmul(
                        out,
                        wt[:, half, mb * 128:mb * 128 + sz],
                        h2_cols[:, kb:kb + 1],
                        start=(kb == 0 and mb in (0, 10)),
                        stop=(kb == KB - 1 and mb in (9, 10)),
                    )
        a_sb = sm.tile([128, 11], F32, name="a_sb")
        sig = sm.tile([128, 11], F32, name="sig")
        # silu(x) = x * sigmoid(x)
        nc.scalar.activation(sig[0:96, 10:11], f1b[:], AF.Sigmoid)
        nc.scalar.activation(sig[:, 0:10], f1a[:], AF.Sigmoid)
        nc.vector.tensor_tensor(a_sb[0:96, 10:11], f1b[:],
                                sig[0:96, 10:11], ALU.mult)
        nc.vector.tensor_tensor(a_sb[:, 0:10], f1a[:],
                                sig[:, 0:10], ALU.mult)

        chunks2 = [pp.tile([1, 512], F32, name=f"c2_{n}", tag="ps")
                   for n in range(8)]
        for kb in range(11):
            sz = FF_KB_SIZES[kb]
            wt = wpool.tile([sz, HIDDEN], F32, name="wf2_t", tag="w")
            nc.sync.dma_start(wt[:], i["wf2"][kb * 128:kb * 128 + sz, :])
            for n in range(8):
                nc.tensor.matmul(
                    chunks2[n][:], a_sb[0:sz, kb:kb + 1],
                    wt[:, n * 512:(n + 1) * 512],
                    start=(kb == 0), stop=False,
                )
        ff_row = sm.tile([1, HIDDEN], F32, name="ff_row", tag="outrow")
        for n in range(8):
            nc.tensor.matmul(
                chunks2[n][:], eighth[:], x2_row[:, n * 512:(n + 1) * 512],
                start=False, stop=True,
            )
            nc.scalar.copy(ff_row[:, n * 512:(n + 1) * 512], chunks2[n][:])

        ar2_in = dram.tile([HIDDEN], F32, name="ar2_in")
        ar2_out = dram.tile([HIDDEN], F32, name="ar2_out")
        nc.sync.dma_start(ar2_in[:], ff_row[:])
        nc.gpsimd.collective_compute(
            "AllReduce", ALU.add,
            replica_groups=[list(range(N_CORES))],
            ins=[ar2_in[:].opt()], outs=[ar2_out[:].opt()],
        )
        nc.sync.dma_start(y[:], ar2_out[:])


_BUILT = None


def _build():
    global _BUILT
    if _BUILT is None:
        nc = bacc.Bacc("TRN2", target_bir_lowering=False, debug=False,
                       num_devices=N_CORES)
        with tile.TileContext(nc) as tc:
            _emit(nc, tc)
        nc.compile()
        _BUILT = nc
    return _BUILT


def _shard(inputs):
    f = lambda a: np.ascontiguousarray(np.asarray(a, dtype=np.float32))
    x = f(inputs["x"])
    attn_norm = f(inputs["attn_norm"])
    ffn_norm = f(inputs["ffn_norm"])
    pos = int(np.asarray(inputs["pos"]))
    sin = f(inputs["sin_cache"][pos])
    cos = f(inputs["cos_cache"][pos])
    wq, wk, wv = f(inputs["w_q"]), f(inputs["w_k"]), f(inputs["w_v"])
    wo, wf1, wf2 = f(inputs["w_o"]), f(inputs["w_ff1"]), f(inputs["w_ff2"])
    kc = f(inputs["k_cache"]).reshape(KV_LEN, N_HEADS * HEAD_DIM)
    vc = f(inputs["v_cache"]).reshape(KV_LEN, N_HEADS * HEAD_DIM)

    in_maps = []
    for c in range(N_CORES):
        qs = slice(c * QKV_N, (c + 1) * QKV_N)
        fs = slice(c * FF_N, (c + 1) * FF_N)
        in_maps.append({
            "x": x,
            "ident32": np.eye(32, dtype=np.float32),
            "attn_norm": attn_norm,
            "ffn_norm": ffn_norm,
            "sin": sin,
            "cos": cos,
            "wq": np.ascontiguousarray(wq[:, qs]),
            "wk": np.ascontiguousarray(wk[:, qs]),
            "wv": np.ascontiguousarray(wv[:, qs]),
            "wo": np.ascontiguousarray(wo[qs, :]),
            "kc": np.ascontiguousarray(kc[:, qs]),
            "vc": np.ascontiguousarray(vc[:, qs]),
            "wf1": np.ascontiguousarray(wf1[:, fs]),
            "wf2": np.ascontiguousarray(wf2[fs, :]),
        })
    return in_maps


def kernel(**inputs):
    nc = _build()
    in_maps = _shard(inputs)
    res = bass_utils.run_bass_kernel_spmd(
        nc, in_maps, core_ids=list(range(N_CORES)))
    return res.results[0]["y"]



# revision 15
# speedup vs baseline: 1.2174x; 1.2174x over previous
"""Llama decode block (single token) on 8 TRN2 NeuronCores, tensor-parallel.

Sharding (per sharding_hint): w_q/w_k/w_v/w_ff1 column-sharded, w_o/w_ff2
row-sharded, KV cache sharded by head (4 heads/core). AllReduce after the
attention output projection and after w_ff2. The residual stream x is folded
into the all-reduces as x/8 per core, so each AR output is the full residual
sum directly.

Dtype strategy (validated numerically against the rel-err metric):
  - attention weights + KV cache in fp16 (e5m10): 2 B/elem halves DMA bytes,
    matmuls run at full PE rate; error lands ~1e-2 on the metric (gate 2e-2)
  - FFN weights/activations as bf16 hi+lo splits (w = hi + lo, three matmul
    passes hi*hi + hi*lo + lo*hi accumulated in one PSUM group): fp32-like
    accuracy at full bf16 PE rate; bytes same as f32
  - residual stream, softmax, rmsnorm, collectives in f32

Per-core dataflow:
  rmsnorm(x) -> h16 cols [128,32] fp16
  qT[128d,4h] via weight-stationary matmuls (wq tiles stationary, h moving)
  scores on PE: host-transposed kcT tiles stationary, qT cols moving,
  per 1024-token super-tile: 32 score matmuls -> one exp -> 32 AV matmuls
  k/v rows for the current token; RoPE; appended analytically
  o @ w_o + x/8 -> AllReduce #1 -> x2
  rmsnorm(x2) -> h2 hi/lo; silu(h2 @ w_ff1) hi/lo -> a hi/lo
  a @ w_ff2 + x2/8 -> AllReduce #2 -> y
"""

import math

import numpy as np

import concourse.bass as bass
import concourse.mybir as mybir
import concourse.tile as tile
from concourse import bacc
from concourse import bass_utils

F32 = mybir.dt.float32
F16 = mybir.dt.float16
BF16 = mybir.dt.bfloat16
AF = mybir.ActivationFunctionType
ALU = mybir.AluOpType

HIDDEN = 4096
N_HEADS = 32
HEAD_DIM = 128
INTERM = 11008
KV_LEN = 4096
N_CORES = 8

HEADS_PC = N_HEADS // N_CORES          # 4 heads per core
QKV_N = HEADS_PC * HEAD_DIM            # 512
FF_N = INTERM // N_CORES               # 1376
KB = HIDDEN // 128                     # 32 k-blocks of the hidden dim
FF_KB_SIZES = [128] * 10 + [96]        # 1376 = 10*128 + 96
SCALE = 1.0 / math.sqrt(HEAD_DIM)


def _emit(nc, tc):
    i = {}

    def din(name, shape, dt=F32):
        i[name] = nc.dram_tensor(name, list(shape), dt, kind="ExternalInput").ap()

    din("x", [HIDDEN])
    din("attn_norm", [HIDDEN])
    din("ffn_norm", [HIDDEN])
    din("sin", [HEAD_DIM // 2])
    din("cos", [HEAD_DIM // 2])
    din("ident32", [32, 32])
    din("wq", [HIDDEN, QKV_N], F16)
    din("wkv", [HIDDEN, 2 * QKV_N], F16)
    din("kcT", [QKV_N, KV_LEN], F16)
    din("vc", [KV_LEN, QKV_N], F16)
    din("wo", [QKV_N, HIDDEN], F16)
    din("wf1h", [HIDDEN, FF_N], BF16)
    din("wf1l", [HIDDEN, FF_N], BF16)
    din("wf2h", [FF_N, HIDDEN], BF16)
    din("wf2l", [FF_N, HIDDEN], BF16)
    y = nc.dram_tensor("y", [HIDDEN], F32, kind="ExternalOutput").ap()

    with (
        tc.tile_pool(name="const", bufs=1) as cpool,
        tc.tile_pool(name="wqp", bufs=2) as wqp,
        tc.tile_pool(name="wkvp", bufs=2) as wkvp,
        tc.tile_pool(name="kp", bufs=2) as kp,
        tc.tile_pool(name="vp", bufs=2) as vp,
        tc.tile_pool(name="wop", bufs=2) as wop,
        tc.tile_pool(name="wf1p", bufs=5) as wf1p,
        tc.tile_pool(name="wf2p", bufs=3) as wf2p,
        tc.tile_pool(name="sm", bufs=1) as sm,
        tc.tile_pool(name="psum", bufs=8, space="PSUM") as pp,
        tc.tile_pool(name="dram", bufs=1, space="DRAM") as dram,
    ):
        # ---- constants ----
        ones32 = cpool.tile([32, 1], F32)
        nc.vector.memset(ones32[:], 1.0)
        ones128 = cpool.tile([128, 1], F32)
        nc.vector.memset(ones128[:], 1.0)
        ones128h = cpool.tile([128, 1], F16)
        nc.vector.memset(ones128h[:], 1.0)
        ones_r32 = cpool.tile([1, 32], F32)
        nc.vector.memset(ones_r32[:], 1.0)
        ones_r128 = cpool.tile([1, 128], F32)
        nc.vector.memset(ones_r128[:], 1.0)
        eighth = cpool.tile([1, 1], F32)
        nc.vector.memset(eighth[:], 1.0 / N_CORES)
        eps11 = cpool.tile([1, 1], F32)
        nc.vector.memset(eps11[:], 1e-6)
        ident1 = cpool.tile([1, 1], F32)
        nc.vector.memset(ident1[:], 1.0)
        ident32 = cpool.tile([32, 32], F32)
        nc.sync.dma_start(ident32[:], i["ident32"])

        # sin/cos as rows (for k RoPE) and columns (for qT RoPE, q-scaled)
        sin_row = cpool.tile([1, 64], F32)
        cos_row = cpool.tile([1, 64], F32)
        nc.sync.dma_start(sin_row[:], i["sin"].rearrange("(a d) -> a d", a=1))
        nc.sync.dma_start(cos_row[:], i["cos"].rearrange("(a d) -> a d", a=1))
        sin_col = cpool.tile([64, 1], F32)
        cos_col = cpool.tile([64, 1], F32)
        nc.sync.dma_start(sin_col[:], i["sin"].rearrange("(d a) -> d a", a=1))
        nc.sync.dma_start(cos_col[:], i["cos"].rearrange("(d a) -> d a", a=1))
        sinq_col = cpool.tile([64, 1], F32)
        cosq_col = cpool.tile([64, 1], F32)
        nc.vector.tensor_scalar_mul(sinq_col[:], sin_col[:], SCALE)
        nc.vector.tensor_scalar_mul(cosq_col[:], cos_col[:], SCALE)

        # ---- rmsnorm -> cols [128, 32] f32 ----
        def rmsnorm_cols(x_dram, norm_dram, tag):
            x_rows = sm.tile([32, 128], F32, name=f"x_rows_{tag}", tag="x_rows")
            nrm_rows = sm.tile([32, 128], F32, name=f"nrm_rows_{tag}", tag="nrm_rows")
            nc.sync.dma_start(x_rows[:], x_dram.rearrange("(a d) -> a d", a=32))
            nc.sync.dma_start(nrm_rows[:], norm_dram.rearrange("(a d) -> a d", a=32))
            sq = sm.tile([32, 128], F32, name=f"sq_{tag}", tag="sq")
            ssq = sm.tile([32, 1], F32, name=f"ssq_{tag}", tag="ssq")
            nc.scalar.activation(sq[:], x_rows[:], AF.Square, accum_out=ssq[:])
            ms_psum = pp.tile([1, 1], F32, name=f"ms_psum_{tag}", tag="ps")
            nc.tensor.matmul(ms_psum[:], ones32[:], ssq[:])
            rstd = sm.tile([1, 1], F32, name=f"rstd_{tag}", tag="rstd")
            nc.scalar.activation(rstd[:], ms_psum[:], AF.Sqrt,
                                 bias=eps11[:], scale=1.0 / HIDDEN)
            nc.vector.reciprocal(rstd[:], rstd[:])
            rstd_ps = pp.tile([32, 1], F32, name=f"rstd_ps_{tag}", tag="ps")
            nc.tensor.matmul(rstd_ps[:], ones_r32[:], rstd[:])
            rstd32 = sm.tile([32, 1], F32, name=f"rstd32_{tag}", tag="rstd32")
            nc.vector.tensor_copy(rstd32[:], rstd_ps[:])
            h_rows = sm.tile([32, 128], F32, name=f"h_rows_{tag}", tag="h_rows")
            nc.vector.tensor_tensor(h_rows[:], x_rows[:], nrm_rows[:], ALU.mult)
            nc.vector.tensor_scalar_mul(h_rows[:], h_rows[:], rstd32[:])
            h_psum = pp.tile([128, 32], F32, name=f"h_psum_{tag}", tag="ps")
            nc.tensor.transpose(h_psum[:], h_rows[:], ident32[:])
            h_cols = sm.tile([128, 32], F32, name=f"h_cols_{tag}", tag="hcols")
            nc.vector.tensor_copy(h_cols[:], h_psum[:])
            return h_cols

        h_cols = rmsnorm_cols(i["x"], i["attn_norm"], "a")
        h16 = sm.tile([128, 32], F16, name="h16")
        nc.vector.tensor_copy(h16[:], h_cols[:])

        # ---- qT[128d, 4h] = (h @ wq) transposed, weight-stationary ----
        qT_ps = pp.tile([128, HEADS_PC], F32, name="qT_ps", tag="ps")
        for t8 in range(8):
            wq_t = wqp.tile([128, 4, QKV_N], F16, name="wq_t", tag="wq")
            nc.sync.dma_start(
                wq_t[:],
                i["wq"][t8 * 512:(t8 + 1) * 512, :].rearrange(
                    "(b p) c -> p b c", p=128),
            )
            for b in range(4):
                kb = t8 * 4 + b
                for h in range(HEADS_PC):
                    nc.tensor.matmul(
                        qT_ps[:, h:h + 1],
                        wq_t[:, b, h * 128:(h + 1) * 128],
                        h16[:, kb:kb + 1],
                        start=(kb == 0 and h == 0),
                        stop=(kb == KB - 1 and h == HEADS_PC - 1),
                        skip_group_check=True,
                    )

        # RoPE on qT columns (q pre-scaled by 1/sqrt(d) via scaled sin/cos)
        qT32 = sm.tile([128, HEADS_PC], F32, name="qT32")
        tmpq = sm.tile([128, HEADS_PC], F32, name="tmpq")
        cb = cosq_col[:].to_broadcast((64, HEADS_PC))
        sb = sinq_col[:].to_broadcast((64, HEADS_PC))
        nc.vector.tensor_tensor(qT32[0:64, :], qT_ps[0:64, :], cb, ALU.mult)
        nc.vector.tensor_tensor(tmpq[0:64, :], qT_ps[64:128, :], sb, ALU.mult)
        nc.vector.tensor_sub(qT32[0:64, :], qT32[0:64, :], tmpq[0:64, :])
        nc.vector.tensor_tensor(qT32[64:128, :], qT_ps[64:128, :], cb, ALU.mult)
        nc.vector.tensor_tensor(tmpq[64:128, :], qT_ps[0:64, :], sb, ALU.mult)
        nc.vector.tensor_add(qT32[64:128, :], qT32[64:128, :], tmpq[64:128, :])
        qT16 = sm.tile([128, HEADS_PC], F16, name="qT16")
        nc.vector.tensor_copy(qT16[:], qT32[:])

        # ---- attention over the KV cache: 4 super-tiles of 1024 tokens ----
        o_ps = pp.tile([128, HEADS_PC], F32, name="o_ps", tag="ps")
        den_acc = sm.tile([1, HEADS_PC], F32, name="den_acc")
        nc.vector.memset(den_acc[:], 0.0)

        for g in range(4):
            kcT_t = kp.tile([128, 4, 1024], F16, name="kcT_t", tag="k")
            nc.sync.dma_start(
                kcT_t[:],
                i["kcT"][:, g * 1024:(g + 1) * 1024].rearrange(
                    "(b p) t -> p b t", p=128),
            )
            v_t = vp.tile([128, 8, QKV_N], F16, name="v_t", tag="v")
            nc.sync.dma_start(
                v_t[:],
                i["vc"][g * 1024:(g + 1) * 1024, :].rearrange(
                    "(b p) c -> p b c", p=128),
            )
            s_ps = pp.tile([128, 32], F32, name="s_ps", tag="ps")
            for tt in range(8):
                for h in range(HEADS_PC):
                    nc.tensor.matmul(
                        s_ps[:, tt * 4 + h:tt * 4 + h + 1],
                        kcT_t[:, h, tt * 128:(tt + 1) * 128],
                        qT16[:, h:h + 1],
                        start=(tt == 0 and h == 0),
                        stop=(tt == 7 and h == HEADS_PC - 1),
                        skip_group_check=True,
                    )
            exp_sb = sm.tile([128, 32], F16, name=f"exp_sb_{g}", tag=f"exp{g % 2}")
            nc.scalar.activation(exp_sb[:], s_ps[:], AF.Exp)
            den_ps = pp.tile([1, 32], F32, name="den_ps", tag="ps")
            nc.tensor.matmul(den_ps[:], ones128h[:], exp_sb[:])
            den_g = sm.tile([1, HEADS_PC], F32, name="den_g", tag="deng")
            nc.vector.tensor_reduce(
                den_g[:],
                den_ps[:].rearrange("a (t h) -> a h t", h=HEADS_PC),
                mybir.AxisListType.X, ALU.add)
            nc.vector.tensor_add(den_acc[:], den_acc[:], den_g[:])
            for tt in range(8):
                for h in range(HEADS_PC):
                    nc.tensor.matmul(
                        o_ps[:, h:h + 1],
                        v_t[:, tt, h * 128:(h + 1) * 128],
                        exp_sb[:, tt * 4 + h:tt * 4 + h + 1],
                        start=(g == 0 and tt == 0 and h == 0), stop=False,
                        skip_group_check=True,
                    )

        # ---- current-token k/v rows ----
        k_ps = pp.tile([1, QKV_N], F32, name="k_ps", tag="ps")
        v_ps = pp.tile([1, QKV_N], F32, name="v_ps", tag="ps")
        for t8 in range(8):
            wkv_t = wkvp.tile([128, 4, 2 * QKV_N], F16, name="wkv_t", tag="wkv")
            nc.sync.dma_start(
                wkv_t[:],
                i["wkv"][t8 * 512:(t8 + 1) * 512, :].rearrange(
                    "(b p) c -> p b c", p=128),
            )
            for b in range(4):
                kb = t8 * 4 + b
                nc.tensor.matmul(k_ps[:], h16[:, kb:kb + 1],
                                 wkv_t[:, b, 0:QKV_N],
                                 start=(kb == 0), stop=(kb == KB - 1))
                nc.tensor.matmul(v_ps[:], h16[:, kb:kb + 1],
                                 wkv_t[:, b, QKV_N:2 * QKV_N],
                                 start=(kb == 0), stop=(kb == KB - 1))

        # RoPE on k row (unscaled)
        k_rot = sm.tile([1, QKV_N], F32, name="k_rot")
        tmpk = sm.tile([1, QKV_N], F32, name="tmpk")
        k3 = k_ps[:].rearrange("a (h d) -> a h d", h=HEADS_PC)
        o3 = k_rot[:].rearrange("a (h d) -> a h d", h=HEADS_PC)
        t3 = tmpk[:].rearrange("a (h d) -> a h d", h=HEADS_PC)
        cbr = cos_row[:].unsqueeze(1).to_broadcast((1, HEADS_PC, 64))
        sbr = sin_row[:].unsqueeze(1).to_broadcast((1, HEADS_PC, 64))
        nc.vector.tensor_tensor(o3[:, :, 0:64], k3[:, :, 0:64], cbr, ALU.mult)
        nc.vector.tensor_tensor(t3[:, :, 0:64], k3[:, :, 64:128], sbr, ALU.mult)
        nc.vector.tensor_sub(o3[:, :, 0:64], o3[:, :, 0:64], t3[:, :, 0:64])
        nc.vector.tensor_tensor(o3[:, :, 64:128], k3[:, :, 64:128], cbr, ALU.mult)
        nc.vector.tensor_tensor(t3[:, :, 64:128], k3[:, :, 0:64], sbr, ALU.mult)
        nc.vector.tensor_add(o3[:, :, 64:128], o3[:, :, 64:128],
                             t3[:, :, 64:128])
        v16_row = sm.tile([1, QKV_N], F16, name="v16_row")
        nc.vector.tensor_copy(v16_row[:], v_ps[:])

        # kT_new columns via PE transposes of the k row
        kTn_ps = pp.tile([128, HEADS_PC], F32, name="kTn_ps", tag="ps")
        for h in range(HEADS_PC):
            nc.tensor.transpose(kTn_ps[:, h:h + 1],
                                k_rot[:, h * 128:(h + 1) * 128], ident1[:])
        prod = sm.tile([128, HEADS_PC], F32, name="prod")
        nc.vector.tensor_tensor(prod[:], qT32[:], kTn_ps[:], ALU.mult)
        s_new_ps = pp.tile([1, HEADS_PC], F32, name="s_new_ps", tag="ps")
        nc.tensor.matmul(s_new_ps[:], ones128[:], prod[:])
        e_new = sm.tile([1, HEADS_PC], F32, name="e_new")
        nc.scalar.activation(e_new[:], s_new_ps[:], AF.Exp)
        nc.vector.tensor_add(den_acc[:], den_acc[:], e_new[:])
        e_new16 = sm.tile([1, HEADS_PC], F16, name="e_new16")
        nc.vector.tensor_copy(e_new16[:], e_new[:])
        for h in range(HEADS_PC):
            nc.tensor.matmul(
                o_ps[:, h:h + 1],
                v16_row[:, h * 128:(h + 1) * 128],
                e_new16[:, h:h + 1],
                start=False, stop=(h == HEADS_PC - 1),
                skip_group_check=True,
            )

        # normalize: o = o_ps / den
        nc.vector.reciprocal(den_acc[:], den_acc[:])
        recip_ps = pp.tile([128, HEADS_PC], F32, name="recip_ps", tag="ps")
        nc.tensor.matmul(recip_ps[:], ones_r128[:], den_acc[:])
        recip_sb = sm.tile([128, HEADS_PC], F32, name="recip_sb")
        nc.vector.tensor_copy(recip_sb[:], recip_ps[:])
        o_sb = sm.tile([128, HEADS_PC], F16, name="o_sb")
        nc.vector.tensor_tensor(o_sb[:], o_ps[:], recip_sb[:], ALU.mult)

        # ---- o @ w_o + x/8 -> [1,4096] -> AllReduce #1 ----
        ar1_in = dram.tile([HIDDEN], F32, name="ar1_in")
        ar1_out = dram.tile([HIDDEN], F32, name="ar1_out")

        chunks1 = [pp.tile([1, 512], F32, name=f"c1_{n}", tag="ps")
                   for n in range(8)]
        for kb in range(HEADS_PC):
            wo_t = wop.tile([128, HIDDEN], F16, name="wo_t", tag="wo")
            nc.sync.dma_start(wo_t[:], i["wo"][kb * 128:(kb + 1) * 128, :])
            for n in range(8):
                nc.tensor.matmul(
                    chunks1[n][:], o_sb[:, kb:kb + 1],
                    wo_t[:, n * 512:(n + 1) * 512],
                    start=(kb == 0), stop=False,
                )
        for n in range(8):
            xch = sm.tile([1, 512], F32, name=f"xr_{n}", tag=f"xr{n % 2}")
            nc.sync.dma_start(
                xch[:], i["x"][n * 512:(n + 1) * 512].rearrange("(a d) -> a d", a=1))
            nc.tensor.matmul(
                chunks1[n][:], eighth[:], xch[:],
                start=False, stop=True,
            )
            orow_c = sm.tile([1, 512], F32, name=f"or_{n}", tag=f"or{n % 2}")
            nc.vector.tensor_copy(orow_c[:], chunks1[n][:])
            nc.sync.dma_start(ar1_in[n * 512:(n + 1) * 512], orow_c[:])
        nc.gpsimd.collective_compute(
            "AllReduce", ALU.add,
            replica_groups=[list(range(N_CORES))],
            ins=[ar1_in[:].opt()], outs=[ar1_out[:].opt()],
        )

        # ---- MLP ----
        h2_cols = rmsnorm_cols(ar1_out[:], i["ffn_norm"], "b")

        # h2 hi/lo interleaved columns [hi0 lo0 hi1 lo1 ...]
        h2hl = sm.tile([128, 64], BF16, name="h2hl")
        h2e = h2hl[:].rearrange("p (k j) -> p k j", j=2)
        h2c3 = h2_cols[:].rearrange("p (k j) -> p k j", j=1)
        nc.vector.tensor_copy(h2e[:, :, 0:1], h2c3)
        h2h32 = sm.tile([128, 32], F32, name="h2h32")
        h2h3 = h2h32[:].rearrange("p (k j) -> p k j", j=1)
        nc.vector.tensor_copy(h2h3, h2e[:, :, 0:1])
        nc.vector.tensor_sub(h2h32[:], h2_cols[:], h2h32[:])
        nc.vector.tensor_copy(h2e[:, :, 1:2], h2h3)

        # wf1: weight-stationary, psum cols (mb, hi/lo-in) pairs
        f1a = pp.tile([128, 20], F32, name="f1a", tag="ps")
        f1b = pp.tile([96, 2], F32, name="f1b", tag="ps")
        for t8 in range(8):
            w1h_t = wf1p.tile([128, 4, FF_N], BF16, name="w1h_t", tag="wf1")
            nc.sync.dma_start(
                w1h_t[:],
                i["wf1h"][t8 * 512:(t8 + 1) * 512, :].rearrange(
                    "(b p) c -> p b c", p=128),
            )
            w1l_t = wf1p.tile([128, 4, FF_N], BF16, name="w1l_t", tag="wf1")
            nc.sync.dma_start(
                w1l_t[:],
                i["wf1l"][t8 * 512:(t8 + 1) * 512, :].rearrange(
                    "(b p) c -> p b c", p=128),
            )
            for b in range(4):
                kb = t8 * 4 + b
                for mb in range(11):
                    sz = FF_KB_SIZES[mb]
                    out = f1a[:, 2 * mb:2 * mb + 2] if mb < 10 else f1b[:]
                    outh = f1a[:, 2 * mb:2 * mb + 1] if mb < 10 else f1b[:, 0:1]
                    nc.tensor.matmul(
                        out,
                        w1h_t[:, b, mb * 128:mb * 128 + sz],
                        h2hl[:, 2 * kb:2 * kb + 2],
                        start=(kb == 0 and mb in (0, 10)), stop=False,
                        skip_group_check=True,
                    )
                    nc.tensor.matmul(
                        outh,
                        w1l_t[:, b, mb * 128:mb * 128 + sz],
                        h2hl[:, 2 * kb:2 * kb + 1],
                        start=False,
                        stop=(kb == KB - 1 and mb in (9, 10)),
                        skip_group_check=True,
                    )

        # pre-activation = hi-col + lo-col; silu; a hi/lo interleaved
        pre_a = sm.tile([128, 11], F32, name="pre_a")
        nc.vector.tensor_reduce(
            pre_a[:, 0:10],
            f1a[:].rearrange("p (m j) -> p m j", j=2),
            mybir.AxisListType.X, ALU.add)
        nc.vector.tensor_reduce(
            pre_a[0:96, 10:11],
            f1b[:].rearrange("p (m j) -> p m j", j=2),
            mybir.AxisListType.X, ALU.add)
        sig = sm.tile([128, 11], F32, name="sig")
        nc.scalar.activation(sig[:], pre_a[:], AF.Sigmoid)
        a32 = sm.tile([128, 11], F32, name="a32")
        nc.vector.tensor_tensor(a32[:], pre_a[:], sig[:], ALU.mult)

        aHL = sm.tile([128, 22], BF16, name="aHL")
        ae = aHL[:].rearrange("p (k j) -> p k j", j=2)
        a3 = a32[:].rearrange("p (k j) -> p k j", j=1)
        nc.vector.tensor_copy(ae[:, :, 0:1], a3)
        ah32 = sm.tile([128, 11], F32, name="ah32")
        ah3 = ah32[:].rearrange("p (k j) -> p k j", j=1)
        nc.vector.tensor_copy(ah3, ae[:, :, 0:1])
        nc.vector.tensor_sub(ah32[:], a32[:], ah32[:])
        nc.vector.tensor_copy(ae[:, :, 1:2], ah3)

        # wf2: a-stationary hi/lo, weights moving; 3 products into one row
        chunks2 = [pp.tile([1, 512], F32, name=f"c2_{n}", tag="ps")
                   for n in range(8)]
        ff2_chunks = [(0, 2, 128), (2, 4, 128), (4, 6, 128), (6, 8, 128),
                      (8, 10, 128), (10, 11, 96)]
        for ci, (kb0, kb1, _) in enumerate(ff2_chunks):
            nk = kb1 - kb0
            rows = sum(FF_KB_SIZES[kb0:kb1])
            w2h_t = wf2p.tile([128, 2, HIDDEN], BF16, name="w2h_t", tag="wf2")
            nc.sync.dma_start(
                w2h_t[0:128 if nk == 2 else 96, 0:nk, :],
                i["wf2h"][kb0 * 128:kb0 * 128 + rows, :].rearrange(
                    "(b p) c -> p b c", b=nk),
            )
            w2l_t = wf2p.tile([128, 2, HIDDEN], BF16, name="w2l_t", tag="wf2")
            nc.sync.dma_start(
                w2l_t[0:128 if nk == 2 else 96, 0:nk, :],
                i["wf2l"][kb0 * 128:kb0 * 128 + rows, :].rearrange(
                    "(b p) c -> p b c", b=nk),
            )
            for j in range(nk):
                kb = kb0 + j
                sz = FF_KB_SIZES[kb]
                for n in range(8):
                    nc.tensor.matmul(
                        chunks2[n][:],
                        aHL[0:sz, 2 * kb:2 * kb + 1],
                        w2h_t[0:sz, j, n * 512:(n + 1) * 512],
                        start=(kb == 0), stop=False,
                        skip_group_check=True,
                    )
                    nc.tensor.matmul(
                        chunks2[n][:],
                        aHL[0:sz, 2 * kb + 1:2 * kb + 2],
                        w2h_t[0:sz, j, n * 512:(n + 1) * 512],
                        start=False, stop=False,
                        skip_group_check=True,
                    )
                    nc.tensor.matmul(
                        chunks2[n][:],
                        aHL[0:sz, 2 * kb:2 * kb + 1],
                        w2l_t[0:sz, j, n * 512:(n + 1) * 512],
                        start=False, stop=False,
                        skip_group_check=True,
                    )

        ar2_in = dram.tile([HIDDEN], F32, name="ar2_in")
        ar2_out = dram.tile([HIDDEN], F32, name="ar2_out")
        for n in range(8):
            x2ch = sm.tile([1, 512], F32, name=f"x2r_{n}", tag=f"xr{n % 2}")
            nc.sync.dma_start(
                x2ch[:],
                ar1_out[n * 512:(n + 1) * 512].rearrange("(a d) -> a d", a=1))
            nc.tensor.matmul(
                chunks2[n][:], eighth[:], x2ch[:],
                start=False, stop=True,
                skip_group_check=True,
            )
            ffc = sm.tile([1, 512], F32, name=f"ff_{n}", tag=f"or{n % 2}")
            nc.vector.tensor_copy(ffc[:], chunks2[n][:])
            nc.sync.dma_start(ar2_in[n * 512:(n + 1) * 512], ffc[:])
        nc.gpsimd.collective_compute(
            "AllReduce", ALU.add,
            replica_groups=[list(range(N_CORES))],
            ins=[ar2_in[:].opt()], outs=[ar2_out[:].opt()],
        )
        nc.sync.dma_start(y[:], ar2_out[:])


_BUILT = None


def _build():
    global _BUILT
    if _BUILT is None:
        nc = bacc.Bacc("TRN2", target_bir_lowering=False, debug=False,
                       num_devices=N_CORES)
        with tile.TileContext(nc) as tc:
            _emit(nc, tc)
        nc.compile()
        _BUILT = nc
    return _BUILT


def _shard(inputs):
    import ml_dtypes  # noqa: F401  (numpy fp16 used; bf16 via ml_dtypes)
    BF = ml_dtypes.bfloat16

    f = lambda a: np.ascontiguousarray(np.asarray(a, dtype=np.float32))
    f16 = lambda a: np.ascontiguousarray(np.asarray(a, dtype=np.float16))

    def hilo(a):
        hi = np.asarray(a, dtype=BF)
        lo = np.asarray(a - hi.astype(np.float32), dtype=BF)
        return np.ascontiguousarray(hi), np.ascontiguousarray(lo)

    x = f(inputs["x"])
    attn_norm = f(inputs["attn_norm"])
    ffn_norm = f(inputs["ffn_norm"])
    pos = int(np.asarray(inputs["pos"]))
    sin = f(inputs["sin_cache"][pos])
    cos = f(inputs["cos_cache"][pos])
    wq, wk, wv = [np.asarray(inputs[k], np.float32) for k in ("w_q", "w_k", "w_v")]
    wo = np.asarray(inputs["w_o"], np.float32)
    wf1 = np.asarray(inputs["w_ff1"], np.float32)
    wf2 = np.asarray(inputs["w_ff2"], np.float32)
    kc = np.asarray(inputs["k_cache"], np.float32).reshape(KV_LEN, N_HEADS * HEAD_DIM)
    vc = np.asarray(inputs["v_cache"], np.float32).reshape(KV_LEN, N_HEADS * HEAD_DIM)

    in_maps = []
    for c in range(N_CORES):
        qs = slice(c * QKV_N, (c + 1) * QKV_N)
        fs = slice(c * FF_N, (c + 1) * FF_N)
        w1h, w1l = hilo(wf1[:, fs])
        w2h, w2l = hilo(wf2[fs, :])
        in_maps.append({
            "x": x,
            "ident32": np.eye(32, dtype=np.float32),
            "attn_norm": attn_norm,
            "ffn_norm": ffn_norm,
            "sin": sin,
            "cos": cos,
            "wq": f16(wq[:, qs]),
            "wkv": f16(np.concatenate([wk[:, qs], wv[:, qs]], axis=1)),
            "kcT": f16(kc[:, qs].T),
            "vc": f16(vc[:, qs]),
            "wo": f16(wo[qs, :]),
            "wf1h": w1h,
            "wf1l": w1l,
            "wf2h": w2h,
            "wf2l": w2l,
        })
    return in_maps


def kernel(**inputs):
    nc = _build()
    in_maps = _shard(inputs)
    res = bass_utils.run_bass_kernel_spmd(
        nc, in_maps, core_ids=list(range(N_CORES)))
    return res.results[0]["y"]


# revision 30
# speedup vs baseline: 1.3395x; 1.1003x over previous
"""Llama decode block (single token) on 8 TRN2 NeuronCores, tensor-parallel.

Sharding (per sharding_hint): w_q/w_k/w_v/w_ff1 column-sharded, w_o/w_ff2
row-sharded, KV cache sharded by head (4 heads/core). AllReduce after the
attention output projection and after w_ff2. The residual stream x is folded
into the all-reduces as x/8 per core, so each AR output is the full residual
sum directly.

Dtype strategy (validated numerically against the rel-err metric):
  - attention weights + KV cache in fp16 (e5m10): 2 B/elem halves DMA bytes,
    matmuls run at full PE rate; error lands ~1e-2 on the metric (gate 2e-2)
  - FFN weights/activations as bf16 hi+lo splits (w = hi + lo, three matmul
    passes hi*hi + hi*lo + lo*hi accumulated in one PSUM group): fp32-like
    accuracy at full bf16 PE rate; bytes same as f32
  - residual stream, softmax, rmsnorm, collectives in f32

Per-core dataflow:
  rmsnorm(x) -> h16 cols [128,32] fp16
  qT[128d,4h] via weight-stationary matmuls (wq tiles stationary, h moving)
  scores on PE: host-transposed kcT tiles stationary, qT cols moving,
  per 1024-token super-tile: 32 score matmuls -> one exp -> 32 AV matmuls
  k/v rows for the current token; RoPE; appended analytically
  o @ w_o + x/8 -> AllReduce #1 -> x2
  rmsnorm(x2) -> h2 hi/lo; silu(h2 @ w_ff1) hi/lo -> a hi/lo
  a @ w_ff2 + x2/8 -> AllReduce #2 -> y
"""

import math

import numpy as np

import concourse.bass as bass
import concourse.mybir as mybir
import concourse.tile as tile
from concourse import bacc
from concourse import bass_utils

F32 = mybir.dt.float32
F16 = mybir.dt.float16
BF16 = mybir.dt.bfloat16
AF = mybir.ActivationFunctionType
ALU = mybir.AluOpType

HIDDEN = 4096
N_HEADS = 32
HEAD_DIM = 128
INTERM = 11008
KV_LEN = 4096
N_CORES = 8

HEADS_PC = N_HEADS // N_CORES          # 4 heads per core
QKV_N = HEADS_PC * HEAD_DIM            # 512
FF_N = INTERM // N_CORES               # 1376
KB = HIDDEN // 128                     # 32 k-blocks of the hidden dim
FF_KB_SIZES = [128] * 10 + [96]        # 1376 = 10*128 + 96
SCALE = 1.0 / math.sqrt(HEAD_DIM)


def _emit(nc, tc):
    i = {}

    def din(name, shape, dt=F32):
        i[name] = nc.dram_tensor(name, list(shape), dt, kind="ExternalInput").ap()

    din("x", [HIDDEN])
    din("attn_norm", [HIDDEN])
    din("ffn_norm", [HIDDEN])
    din("sin", [HEAD_DIM // 2])
    din("cos", [HEAD_DIM // 2])
    din("ident32", [32, 32])
    din("wqkv", [HIDDEN, 3 * QKV_N], F16)
    din("kcT", [QKV_N, KV_LEN], F16)
    din("vc", [KV_LEN, QKV_N], F16)
    din("wo", [QKV_N, HIDDEN], F16)
    din("wf1h", [HIDDEN, FF_N], BF16)
    din("wf1l", [HIDDEN, FF_N], BF16)
    din("wf2h", [FF_N, HIDDEN], BF16)
    din("wf2l", [FF_N, HIDDEN], BF16)
    y = nc.dram_tensor("y", [HIDDEN], F32, kind="ExternalOutput").ap()

    with (
        tc.tile_pool(name="const", bufs=1) as cpool,
        tc.tile_pool(name="wkvp", bufs=2) as wkvp,
        tc.tile_pool(name="kp", bufs=2) as kp,
        tc.tile_pool(name="vp", bufs=2) as vp,
        tc.tile_pool(name="wop", bufs=2) as wop,
        tc.tile_pool(name="wf1p", bufs=5) as wf1p,
        tc.tile_pool(name="wf2p", bufs=3) as wf2p,
        tc.tile_pool(name="sm", bufs=1) as sm,
        tc.tile_pool(name="psum", bufs=8, space="PSUM") as pp,
        tc.tile_pool(name="dram", bufs=1, space="DRAM") as dram,
    ):
        # ---- constants ----
        ones32 = cpool.tile([32, 1], F32)
        nc.vector.memset(ones32[:], 1.0)
        ones128h = cpool.tile([128, 1], F16)
        nc.vector.memset(ones128h[:], 1.0)
        ones_r32 = cpool.tile([1, 32], F32)
        nc.vector.memset(ones_r32[:], 1.0)
        ones_r128 = cpool.tile([1, 128], F32)
        nc.vector.memset(ones_r128[:], 1.0)
        eighth = cpool.tile([1, 1], F32)
        nc.vector.memset(eighth[:], 1.0 / N_CORES)
        eps11 = cpool.tile([1, 1], F32)
        nc.vector.memset(eps11[:], 1e-6)
        ident1 = cpool.tile([1, 1], F32)
        nc.vector.memset(ident1[:], 1.0)
        ident1h = cpool.tile([1, 1], BF16)
        nc.vector.memset(ident1h[:], 1.0)
        ident32 = cpool.tile([32, 32], F32)
        nc.sync.dma_start(ident32[:], i["ident32"])

        # sin/cos as rows (for k RoPE) and columns (for qT RoPE, q-scaled)
        sin_row = cpool.tile([1, 64], F32)
        cos_row = cpool.tile([1, 64], F32)
        nc.sync.dma_start(sin_row[:], i["sin"].rearrange("(a d) -> a d", a=1))
        nc.sync.dma_start(cos_row[:], i["cos"].rearrange("(a d) -> a d", a=1))
        sinq_row = cpool.tile([1, 64], F32)
        cosq_row = cpool.tile([1, 64], F32)
        nc.vector.tensor_scalar_mul(sinq_row[:], sin_row[:], SCALE)
        nc.vector.tensor_scalar_mul(cosq_row[:], cos_row[:], SCALE)

        # ---- rmsnorm -> cols [128, 32] f32 ----
        def rmsnorm_cols(x_dram, norm_dram, tag):
            x_rows = sm.tile([32, 128], F32, name=f"x_rows_{tag}", tag="x_rows")
            nrm_rows = sm.tile([32, 128], F32, name=f"nrm_rows_{tag}", tag="nrm_rows")
            nc.sync.dma_start(x_rows[:], x_dram.rearrange("(a d) -> a d", a=32))
            nc.sync.dma_start(nrm_rows[:], norm_dram.rearrange("(a d) -> a d", a=32))
            sq = sm.tile([32, 128], F32, name=f"sq_{tag}", tag="sq")
            ssq = sm.tile([32, 1], F32, name=f"ssq_{tag}", tag="ssq")
            nc.scalar.activation(sq[:], x_rows[:], AF.Square, accum_out=ssq[:])
            ms_psum = pp.tile([1, 1], F32, name=f"ms_psum_{tag}", tag="ps")
            nc.tensor.matmul(ms_psum[:], ones32[:], ssq[:])
            rstd = sm.tile([1, 1], F32, name=f"rstd_{tag}", tag="rstd")
            nc.scalar.activation(rstd[:], ms_psum[:], AF.Sqrt,
                                 bias=eps11[:], scale=1.0 / HIDDEN)
            nc.vector.reciprocal(rstd[:], rstd[:])
            rstd_ps = pp.tile([32, 1], F32, name=f"rstd_ps_{tag}", tag="ps")
            nc.tensor.matmul(rstd_ps[:], ones_r32[:], rstd[:])
            rstd32 = sm.tile([32, 1], F32, name=f"rstd32_{tag}", tag="rstd32")
            nc.vector.tensor_copy(rstd32[:], rstd_ps[:])
            h_rows = sm.tile([32, 128], F32, name=f"h_rows_{tag}", tag="h_rows")
            nc.vector.tensor_tensor(h_rows[:], x_rows[:], nrm_rows[:], ALU.mult)
            nc.vector.tensor_scalar_mul(h_rows[:], h_rows[:], rstd32[:])
            h_psum = pp.tile([128, 32], F32, name=f"h_psum_{tag}", tag="ps")
            nc.tensor.transpose(h_psum[:], h_rows[:], ident32[:])
            h_cols = sm.tile([128, 32], F32, name=f"h_cols_{tag}", tag="hcols")
            nc.vector.tensor_copy(h_cols[:], h_psum[:])
            return h_cols

        h_cols = rmsnorm_cols(i["x"], i["attn_norm"], "a")
        h16 = sm.tile([128, 32], F16, name="h16")
        nc.vector.tensor_copy(h16[:], h_cols[:])

        # ---- fused q/k/v rows: h @ [wq|wk|wv], weights moving ----
        q_ps = pp.tile([1, QKV_N], F32, name="q_ps", tag="ps")
        k_ps = pp.tile([1, QKV_N], F32, name="k_ps", tag="ps")
        v_ps = pp.tile([1, QKV_N], F32, name="v_ps", tag="ps")
        for t8 in range(8):
            wkv_t = wkvp.tile([128, 4, 3 * QKV_N], F16, name="wkv_t", tag="wkv")
            nc.sync.dma_start(
                wkv_t[:],
                i["wqkv"][t8 * 512:(t8 + 1) * 512, :].rearrange(
                    "(b p) c -> p b c", p=128),
            )
            for b in range(4):
                kb = t8 * 4 + b
                nc.tensor.matmul(q_ps[:], h16[:, kb:kb + 1],
                                 wkv_t[:, b, 0:QKV_N],
                                 start=(kb == 0), stop=(kb == KB - 1))
                nc.tensor.matmul(k_ps[:], h16[:, kb:kb + 1],
                                 wkv_t[:, b, QKV_N:2 * QKV_N],
                                 start=(kb == 0), stop=(kb == KB - 1))
                nc.tensor.matmul(v_ps[:], h16[:, kb:kb + 1],
                                 wkv_t[:, b, 2 * QKV_N:3 * QKV_N],
                                 start=(kb == 0), stop=(kb == KB - 1))

        def rope_row(src_ps, cos_t, sin_t, tag):
            out = sm.tile([1, QKV_N], F32, name=f"rope_{tag}")
            tmp = sm.tile([1, QKV_N], F32, name=f"rope_tmp_{tag}")
            r3 = src_ps[:].rearrange("a (h d) -> a h d", h=HEADS_PC)
            o3 = out[:].rearrange("a (h d) -> a h d", h=HEADS_PC)
            t3 = tmp[:].rearrange("a (h d) -> a h d", h=HEADS_PC)
            cb = cos_t[:].unsqueeze(1).to_broadcast((1, HEADS_PC, 64))
            sb = sin_t[:].unsqueeze(1).to_broadcast((1, HEADS_PC, 64))
            nc.vector.tensor_tensor(o3[:, :, 0:64], r3[:, :, 0:64], cb, ALU.mult)
            nc.vector.tensor_tensor(t3[:, :, 0:64], r3[:, :, 64:128], sb, ALU.mult)
            nc.vector.tensor_sub(o3[:, :, 0:64], o3[:, :, 0:64], t3[:, :, 0:64])
            nc.vector.tensor_tensor(o3[:, :, 64:128], r3[:, :, 64:128], cb, ALU.mult)
            nc.vector.tensor_tensor(t3[:, :, 64:128], r3[:, :, 0:64], sb, ALU.mult)
            nc.vector.tensor_add(o3[:, :, 64:128], o3[:, :, 64:128],
                                 t3[:, :, 64:128])
            return out

        q_rot = rope_row(q_ps, cosq_row, sinq_row, "q")  # pre-scaled
        qT_ps = pp.tile([128, HEADS_PC], F32, name="qT_ps", tag="ps")
        for h in range(HEADS_PC):
            nc.tensor.transpose(qT_ps[:, h:h + 1],
                                q_rot[:, h * 128:(h + 1) * 128], ident1[:])
        qT16 = sm.tile([128, HEADS_PC], F16, name="qT16")
        nc.vector.tensor_copy(qT16[:], qT_ps[:])

        # ---- attention over the KV cache: 4 super-tiles of 1024 tokens ----
        o_ps = pp.tile([128, HEADS_PC], F32, name="o_ps", tag="ps")
        den_acc = sm.tile([1, HEADS_PC], F32, name="den_acc")
        nc.vector.memset(den_acc[:], 0.0)

        for g in range(4):
            kcT_t = kp.tile([128, 4, 1024], F16, name="kcT_t", tag="k")
            nc.sync.dma_start(
                kcT_t[:],
                i["kcT"][:, g * 1024:(g + 1) * 1024].rearrange(
                    "(b p) t -> p b t", p=128),
            )
            v_t = vp.tile([128, 8, QKV_N], F16, name="v_t", tag="v")
            nc.sync.dma_start(
                v_t[:],
                i["vc"][g * 1024:(g + 1) * 1024, :].rearrange(
                    "(b p) c -> p b c", p=128),
            )
            s_ps = pp.tile([128, 32], F32, name="s_ps", tag="ps")
            for tt in range(8):
                for h in range(HEADS_PC):
                    nc.tensor.matmul(
                        s_ps[:, tt * 4 + h:tt * 4 + h + 1],
                        kcT_t[:, h, tt * 128:(tt + 1) * 128],
                        qT16[:, h:h + 1],
                        start=(tt == 0 and h == 0),
                        stop=(tt == 7 and h == HEADS_PC - 1),
                        skip_group_check=True,
                    )
            exp_sb = sm.tile([128, 32], F16, name=f"exp_sb_{g}", tag=f"exp{g % 2}")
            nc.scalar.activation(exp_sb[:], s_ps[:], AF.Exp)
            den_ps = pp.tile([1, 32], F32, name="den_ps", tag="ps")
            nc.tensor.matmul(den_ps[:], ones128h[:], exp_sb[:])
            den_g = sm.tile([1, HEADS_PC], F32, name="den_g", tag="deng")
            nc.vector.tensor_reduce(
                den_g[:],
                den_ps[:].rearrange("a (t h) -> a h t", h=HEADS_PC),
                mybir.AxisListType.X, ALU.add)
            nc.vector.tensor_add(den_acc[:], den_acc[:], den_g[:])
            for tt in range(8):
                for h in range(HEADS_PC):
                    nc.tensor.matmul(
                        o_ps[:, h:h + 1],
                        v_t[:, tt, h * 128:(h + 1) * 128],
                        exp_sb[:, tt * 4 + h:tt * 4 + h + 1],
                        start=(g == 0 and tt == 0 and h == 0), stop=False,
                        skip_group_check=True,
                    )

        # ---- current-token contribution ----
        k_rot = rope_row(k_ps, cos_row, sin_row, "k")  # unscaled
        v16_row = sm.tile([1, QKV_N], F16, name="v16_row")
        nc.vector.tensor_copy(v16_row[:], v_ps[:])

        scr_new = sm.tile([1, QKV_N], F32, name="scr_new")
        nc.vector.tensor_tensor(scr_new[:], q_rot[:], k_rot[:], ALU.mult)
        s_new = sm.tile([1, HEADS_PC], F32, name="s_new")
        nc.vector.tensor_reduce(
            s_new[:],
            scr_new[:].rearrange("a (h d) -> a h d", h=HEADS_PC),
            mybir.AxisListType.X, ALU.add)
        e_new = sm.tile([1, HEADS_PC], F32, name="e_new")
        nc.scalar.activation(e_new[:], s_new[:], AF.Exp)
        nc.vector.tensor_add(den_acc[:], den_acc[:], e_new[:])
        e_new16 = sm.tile([1, HEADS_PC], F16, name="e_new16")
        nc.vector.tensor_copy(e_new16[:], e_new[:])
        for h in range(HEADS_PC):
            nc.tensor.matmul(
                o_ps[:, h:h + 1],
                v16_row[:, h * 128:(h + 1) * 128],
                e_new16[:, h:h + 1],
                start=False, stop=(h == HEADS_PC - 1),
                skip_group_check=True,
            )

        # normalize: o = o_ps / den
        nc.vector.reciprocal(den_acc[:], den_acc[:])
        recip_ps = pp.tile([128, HEADS_PC], F32, name="recip_ps", tag="ps")
        nc.tensor.matmul(recip_ps[:], ones_r128[:], den_acc[:])
        recip_sb = sm.tile([128, HEADS_PC], F32, name="recip_sb")
        nc.vector.tensor_copy(recip_sb[:], recip_ps[:])
        o_sb = sm.tile([128, HEADS_PC], F16, name="o_sb")
        nc.vector.tensor_tensor(o_sb[:], o_ps[:], recip_sb[:], ALU.mult)

        # ---- o @ w_o + x/8 -> [1,4096] -> AllReduce #1 ----
        ar1_in = dram.tile([HIDDEN], F32, name="ar1_in")
        ar1_out = dram.tile([HIDDEN], F32, name="ar1_out")

        chunks1 = [pp.tile([1, 512], F32, name=f"c1_{n}", tag="ps")
                   for n in range(8)]
        for kb in range(HEADS_PC):
            wo_t = wop.tile([128, HIDDEN], F16, name="wo_t", tag="wo")
            nc.sync.dma_start(wo_t[:], i["wo"][kb * 128:(kb + 1) * 128, :])
            for n in range(8):
                nc.tensor.matmul(
                    chunks1[n][:], o_sb[:, kb:kb + 1],
                    wo_t[:, n * 512:(n + 1) * 512],
                    start=(kb == 0), stop=False,
                )
        for n in range(8):
            xch = sm.tile([1, 512], F32, name=f"xr_{n}", tag=f"xr{n % 2}")
            nc.sync.dma_start(
                xch[:], i["x"][n * 512:(n + 1) * 512].rearrange("(a d) -> a d", a=1))
            nc.tensor.matmul(
                chunks1[n][:], eighth[:], xch[:],
                start=False, stop=True,
            )
            orow_c = sm.tile([1, 512], F32, name=f"or_{n}", tag=f"or{n % 2}")
            nc.vector.tensor_copy(orow_c[:], chunks1[n][:])
            nc.sync.dma_start(ar1_in[n * 512:(n + 1) * 512], orow_c[:])
        nc.gpsimd.collective_compute(
            "AllReduce", ALU.add,
            replica_groups=[list(range(N_CORES))],
            ins=[ar1_in[:].opt()], outs=[ar1_out[:].opt()],
        )

        # ---- MLP ----
        h2_cols = rmsnorm_cols(ar1_out[:], i["ffn_norm"], "b")

        # h2 hi/lo bf16; s1[kb] = [h2h | 0*31 | h2l] stationaries (M=33)
        h2h = sm.tile([128, 32], BF16, name="h2h")
        nc.vector.tensor_copy(h2h[:], h2_cols[:])
        h2h32 = sm.tile([128, 32], F32, name="h2h32")
        nc.vector.tensor_copy(h2h32[:], h2h[:])
        nc.vector.tensor_sub(h2h32[:], h2_cols[:], h2h32[:])
        s1 = sm.tile([128, 32, 33], BF16, name="s1")
        nc.vector.memset(s1[:], 0.0)
        h2c3 = h2h[:].rearrange("p (k j) -> p k j", j=1)
        l2c3 = h2h32[:].rearrange("p (k j) -> p k j", j=1)
        nc.vector.tensor_copy(s1[:, :, 0:1], h2c3)
        nc.vector.tensor_copy(s1[:, :, 32:33], l2c3)

        # wf1: h2-stationary (M=33: hi-part row 0, lo-part row 32), w moving
        FF1_CH = [(0, 512), (512, 1024), (1024, 1376)]
        pre_ps = [pp.tile([33, c1 - c0], F32, name=f"pre_{ci}", tag="ps")
                  for ci, (c0, c1) in enumerate(FF1_CH)]
        for t8 in range(8):
            w1h_t = wf1p.tile([128, 4, FF_N], BF16, name="w1h_t", tag="wf1")
            nc.sync.dma_start(
                w1h_t[:],
                i["wf1h"][t8 * 512:(t8 + 1) * 512, :].rearrange(
                    "(b p) c -> p b c", p=128),
            )
            w1l_t = wf1p.tile([128, 4, FF_N], BF16, name="w1l_t", tag="wf1")
            nc.sync.dma_start(
                w1l_t[:],
                i["wf1l"][t8 * 512:(t8 + 1) * 512, :].rearrange(
                    "(b p) c -> p b c", p=128),
            )
            for b in range(4):
                kb = t8 * 4 + b
                for ci, (c0, c1) in enumerate(FF1_CH):
                    nc.tensor.matmul(
                        pre_ps[ci][:],
                        s1[:, kb, :],
                        w1h_t[:, b, c0:c1],
                        start=(kb == 0), stop=False,
                        skip_group_check=True,
                    )
                    nc.tensor.matmul(
                        pre_ps[ci][0:1, :],
                        h2h[:, kb:kb + 1],
                        w1l_t[:, b, c0:c1],
                        start=False, stop=(kb == KB - 1),
                        skip_group_check=True,
                    )

        # pre = row0 + row32; silu on the row; a -> hi/lo rows
        pre_row = sm.tile([1, FF_N], F32, name="pre_row")
        for ci, (c0, c1) in enumerate(FF1_CH):
            pc = sm.tile([1, 512], F32, name=f"pc_{ci}", tag=f"pc{ci % 2}")
            nc.vector.tensor_copy(pc[:, 0:c1 - c0], pre_ps[ci][32:33, :])
            nc.vector.tensor_copy(pre_row[:, c0:c1], pre_ps[ci][0:1, :])
            nc.vector.tensor_tensor(pre_row[:, c0:c1], pre_row[:, c0:c1],
                                    pc[:, 0:c1 - c0], ALU.add)
        sig_row = sm.tile([1, FF_N], F32, name="sig_row", tag="row32a")
        nc.scalar.activation(sig_row[:], pre_row[:], AF.Sigmoid)
        a_row = pre_row  # in-place: a = pre * sigmoid(pre)
        nc.vector.tensor_tensor(a_row[:], pre_row[:], sig_row[:], ALU.mult)
        ah_row = sm.tile([1, FF_N], BF16, name="ah_row")
        nc.vector.tensor_copy(ah_row[:], a_row[:])
        ah32_row = sm.tile([1, FF_N], F32, name="ah32_row", tag="row32b")
        nc.vector.tensor_copy(ah32_row[:], ah_row[:])
        nc.vector.tensor_sub(ah32_row[:], a_row[:], ah32_row[:])
        al_row = sm.tile([1, FF_N], BF16, name="al_row")
        nc.vector.tensor_copy(al_row[:], ah32_row[:])

        # transpose a rows to columns (even cols: 4B-aligned PSUM writes);
        # build s2[kb] = [a_hi | 0*31 | a_lo]
        aT_ps = pp.tile([128, 44], BF16, name="aT_ps", tag="ps")
        for kb in range(11):
            sz = FF_KB_SIZES[kb]
            nc.tensor.transpose(aT_ps[0:sz, 2 * kb:2 * kb + 1],
                                ah_row[:, kb * 128:kb * 128 + sz], ident1h[:])
            nc.tensor.transpose(aT_ps[0:sz, 22 + 2 * kb:23 + 2 * kb],
                                al_row[:, kb * 128:kb * 128 + sz], ident1h[:])
        s2 = sm.tile([128, 11, 33], BF16, name="s2")
        nc.vector.memset(s2[:], 0.0)
        aTh3 = aT_ps[:, 0:22].rearrange("p (k j) -> p k j", j=2)
        aTl3 = aT_ps[:, 22:44].rearrange("p (k j) -> p k j", j=2)
        nc.vector.tensor_copy(s2[:, :, 0:1], aTh3[:, :, 0:1])
        nc.vector.tensor_copy(s2[:, :, 32:33], aTl3[:, :, 0:1])

        # wf2: a-stationary (M=33), weights moving, two passes
        chunks2 = [pp.tile([33, 512], F32, name=f"c2_{n}", tag="ps")
                   for n in range(8)]
        for kb in range(11):
            sz = FF_KB_SIZES[kb]
            w2h_t = wf2p.tile([128, HIDDEN], BF16, name="w2h_t", tag="wf2")
            nc.sync.dma_start(
                w2h_t[0:sz, :], i["wf2h"][kb * 128:kb * 128 + sz, :])
            w2l_t = wf2p.tile([128, HIDDEN], BF16, name="w2l_t", tag="wf2")
            nc.sync.dma_start(
                w2l_t[0:sz, :], i["wf2l"][kb * 128:kb * 128 + sz, :])
            for n in range(8):
                nc.tensor.matmul(
                    chunks2[n][:],
                    s2[0:sz, kb, :],
                    w2h_t[0:sz, n * 512:(n + 1) * 512],
                    start=(kb == 0), stop=False,
                    skip_group_check=True,
                )
                nc.tensor.matmul(
                    chunks2[n][0:1, :],
                    s2[0:sz, kb, 0:1],
                    w2l_t[0:sz, n * 512:(n + 1) * 512],
                    start=False, stop=False,
                    skip_group_check=True,
                )

        ar2_in = dram.tile([HIDDEN], F32, name="ar2_in")
        ar2_out = dram.tile([HIDDEN], F32, name="ar2_out")
        for n in range(8):
            x2ch = sm.tile([1, 512], F32, name=f"x2r_{n}", tag=f"xr{n % 2}")
            nc.sync.dma_start(
                x2ch[:],
                ar1_out[n * 512:(n + 1) * 512].rearrange("(a d) -> a d", a=1))
            nc.tensor.matmul(
                chunks2[n][0:1, :], eighth[:], x2ch[:],
                start=False, stop=True,
                skip_group_check=True,
            )
            c2sb = sm.tile([1, 512], F32, name=f"c2sb_{n}", tag=f"pc{n % 2}")
            nc.vector.tensor_copy(c2sb[:], chunks2[n][32:33, :])
            ffc = sm.tile([1, 512], F32, name=f"ff_{n}", tag=f"or{n % 2}")
            nc.vector.tensor_copy(ffc[:], chunks2[n][0:1, :])
            nc.vector.tensor_tensor(ffc[:], ffc[:], c2sb[:], ALU.add)
            nc.sync.dma_start(ar2_in[n * 512:(n + 1) * 512], ffc[:])
        nc.gpsimd.collective_compute(
            "AllReduce", ALU.add,
            replica_groups=[list(range(N_CORES))],
            ins=[ar2_in[:].opt()], outs=[ar2_out[:].opt()],
        )
        nc.sync.dma_start(y[:], ar2_out[:])


_BUILT = None


def _build():
    global _BUILT
    if _BUILT is None:
        nc = bacc.Bacc("TRN2", target_bir_lowering=False, debug=False,
                       num_devices=N_CORES)
        with tile.TileContext(nc) as tc:
            _emit(nc, tc)
        nc.compile()
        _BUILT = nc
    return _BUILT


def _shard(inputs):
    import ml_dtypes  # noqa: F401  (numpy fp16 used; bf16 via ml_dtypes)
    BF = ml_dtypes.bfloat16

    f = lambda a: np.ascontiguousarray(np.asarray(a, dtype=np.float32))
    f16 = lambda a: np.ascontiguousarray(np.asarray(a, dtype=np.float16))

    def hilo(a):
        hi = np.asarray(a, dtype=BF)
        lo = np.asarray(a - hi.astype(np.float32), dtype=BF)
        return np.ascontiguousarray(hi), np.ascontiguousarray(lo)

    x = f(inputs["x"])
    attn_norm = f(inputs["attn_norm"])
    ffn_norm = f(inputs["ffn_norm"])
    pos = int(np.asarray(inputs["pos"]))
    sin = f(inputs["sin_cache"][pos])
    cos = f(inputs["cos_cache"][pos])
    wq, wk, wv = [np.asarray(inputs[k], np.float32) for k in ("w_q", "w_k", "w_v")]
    wo = np.asarray(inputs["w_o"], np.float32)
    wf1 = np.asarray(inputs["w_ff1"], np.float32)
    wf2 = np.asarray(inputs["w_ff2"], np.float32)
    kc = np.asarray(inputs["k_cache"], np.float32).reshape(KV_LEN, N_HEADS * HEAD_DIM)
    vc = np.asarray(inputs["v_cache"], np.float32).reshape(KV_LEN, N_HEADS * HEAD_DIM)

    in_maps = []
    for c in range(N_CORES):
        qs = slice(c * QKV_N, (c + 1) * QKV_N)
        fs = slice(c * FF_N, (c + 1) * FF_N)
        w1h, w1l = hilo(wf1[:, fs])
        w2h, w2l = hilo(wf2[fs, :])
        in_maps.append({
            "x": x,
            "ident32": np.eye(32, dtype=np.float32),
            "attn_norm": attn_norm,
            "ffn_norm": ffn_norm,
            "sin": sin,
            "cos": cos,
            "wqkv": f16(np.concatenate([wq[:, qs], wk[:, qs], wv[:, qs]], axis=1)),
            "kcT": f16(kc[:, qs].T),
            "vc": f16(vc[:, qs]),
            "wo": f16(wo[qs, :]),
            "wf1h": w1h,
            "wf1l": w1l,
            "wf2h": w2h,
            "wf2l": w2l,
        })
    return in_maps


def kernel(**inputs):
    nc = _build()
    in_maps = _shard(inputs)
    res = bass_utils.run_bass_kernel_spmd(
        nc, in_maps, core_ids=list(range(N_CORES)))
    return res.results[0]["y"]
